# revision 10
# baseline (speedup 1.0000x reference)
"""Trainium2 Bass kernel for a dense pre-norm transformer block (v2: K/V all-gather).

Reference computation (fp32):
    h = LN1(x); qkv = h @ qkv_w + qkv_b; attention (16 heads, no 1/sqrt(d));
    x = x + attn_out @ proj_w + proj_b;
    h2 = LN2(x); x = x + gelu_exact(h2 @ fc1_w + fc1_b) @ fc2_w + fc2_b

Shapes: x [2, 2048, 1024], heads 16 x 64, MLP 4096.

Sharding (8 NeuronCores):
    cores 0-3 -> batch 0, cores 4-7 -> batch 1. Each core owns 512 query
    tokens (x input = just those). LN1 + Q/K/V are computed for the OWN 512
    tokens only; K and V (bf16, 2MB) are then AllGather'd across the 4 cores
    of the batch (staged through HBM). Keys land in global token order --
    attention is permutation-invariant over keys so no rotation is needed;
    the program is SPMD-uniform because even the own K/V block is read back
    from the gather output. Attention/proj/MLP run on the own 512 queries.

Design notes (carried from v1):
    - activations feature-major [C, tokens] on-chip; GEMM operands bf16.
    - LN stats via DVE sums + GpSimd partition_all_reduce; softmax 1/sum via
      GpSimd partition_broadcast.
    - softmax skips max-subtraction (scores ~N(0,3.3^2), fp32 exp safe).
    - K bias dropped (softmax-invariant); V bias folded into proj bias.
"""

import sys

if "/opt/trn_rl_repo" not in sys.path:
    sys.path.insert(0, "/opt/trn_rl_repo")

import numpy as np

import concourse.bass as bass
import concourse.bass_isa as bass_isa
import concourse.mybir as mybir
import concourse.tile as tile
from concourse import bacc
from concourse.bass_utils import run_bass_kernel_spmd

F32 = mybir.dt.float32
F32R = mybir.dt.float32r
BF16 = mybir.dt.bfloat16
AF = mybir.ActivationFunctionType
ALU = mybir.AluOpType

DIM = 1024
CT = DIM // 128          # 8 feature tiles
NTOK = 2048              # tokens per batch
NQ = 512                 # query tokens per core
H = 16
D = 64
MLP = 4096
FT = MLP // 128          # 32 mlp feature tiles
EPS = 1e-5
N_CORES = 8
GELU_AF = None
NGRP = 4                 # attention processed in 4 groups of 2 head-pairs
GP = 2                   # head pairs per group (256 qkv columns)
GW = GP * 128            # qkv columns per group


def _dma(nc, out, in_):
    nc.sync.dma_start(out=out, in_=in_)


def _ln_stats_rep(nc, pool, ar_sum, ar_sq, ntok_norm, tag):
    """From all-reduced (partition-replicated) sum / sum-of-squares tiles
    [128, NQ] produce replicated rstd and mean*rstd tiles [128, NQ]."""
    mean = pool.tile([128, NQ], F32, tag=f"mean{tag}", bufs=1, name="mean")
    m2 = pool.tile([128, NQ], F32, tag=f"m2{tag}", bufs=1, name="m2")
    nc.scalar.activation(mean[:], ar_sum[:], AF.Identity, scale=1.0 / ntok_norm)
    nc.scalar.activation(m2[:], mean[:], AF.Square)
    nc.vector.scalar_tensor_tensor(
        m2[:], ar_sq[:], 1.0 / ntok_norm, m2[:],
        op0=ALU.mult, op1=ALU.subtract)
    nc.vector.tensor_scalar_add(m2[:], m2[:], EPS)
    nc.vector.reciprocal(m2[:], m2[:])
    rstd = pool.tile([128, NQ], BF16, tag=f"rstd{tag}", bufs=2, name="rstd")
    mrs = pool.tile([128, NQ], BF16, tag=f"mrs{tag}", bufs=2, name="mrs")
    nc.scalar.activation(rstd[:], m2[:], AF.Sqrt)
    nc.vector.tensor_mul(mrs[:], mean[:], rstd[:])
    return rstd, mrs


def build_program(reps=1):
    nc = bacc.Bacc("TRN2", target_bir_lowering=False)

    xT = nc.declare_dram_parameter("xT", [DIM, NQ], F32R, isOutput=False)
    qkv_wb = nc.declare_dram_parameter("qkv_wb", [DIM, 3 * DIM], BF16, isOutput=False)
    proj_wb = nc.declare_dram_parameter("proj_wb", [DIM, DIM], BF16, isOutput=False)
    fc1_wb = nc.declare_dram_parameter("fc1_wb", [DIM, MLP], BF16, isOutput=False)
    fc2_wb = nc.declare_dram_parameter("fc2_wb", [MLP, DIM], BF16, isOutput=False)
    bias_pack = nc.declare_dram_parameter("bias_pack", [128, 104], F32, isOutput=False)
    outT = nc.declare_dram_parameter("outT", [DIM, NQ], F32, isOutput=True)

    with tile.TileContext(nc, pool_alloc_mode="queue") as tc:
      for _rep in range(reps):
            with (
                tc.tile_pool(name="const", bufs=1) as const,
                tc.tile_pool(name="xres", bufs=CT) as xres_pool,
                tc.tile_pool(name="yT", bufs=CT) as yT_pool,
                tc.tile_pool(name="stat", bufs=1) as sb_stat,
                tc.tile_pool(name="dram", bufs=1, space="DRAM") as dram,
            ):
                bp = const.tile([128, 104], F32, tag="bp")
                _dma(nc, bp[:], bias_pack[:, :])
                ln1g_t = bp[:, 0:8]
                ln1b_t = bp[:, 8:16]
                ln2g_t = bp[:, 16:24]
                ln2b_t = bp[:, 24:32]
                projb_t = bp[:, 32:40]
                fc2b_t = bp[:, 40:48]
                fc1b_t = bp[:, 48:80]
                qb_q = bp[:, 80:88]

                xres = [xres_pool.tile([128, NQ], F32R, tag="xres",
                                       name=f"xres{i}") for i in range(CT)]
                yT = [yT_pool.tile([128, NQ], BF16, tag="yT", name=f"yT{i}")
                      for i in range(CT)]

                # staging DRAM for the K/V all-gather, split in two halves
                # (groups 0-1 / groups 2-3) so attention on the first half
                # overlaps the second collective. Per half: cols [0:2048) K
                # ([128, GP, 512] per group), [2048:4096) V ([128,4,4,64]).
                kv_in = [dram.tile([128, 4096], BF16, name=f"kv_in{h}")
                         for h in range(2)]
                kv_out = [dram.tile([4, 128, 4096], BF16, name=f"kv_out{h}")
                          for h in range(2)]

                with (
                    tc.tile_pool(name="kvp", bufs=1) as kv_pool,
                ):
                    kTm = [kv_pool.tile([128, GP, NTOK], BF16, tag=f"kTm{g}",
                                        name=f"kTm{g}") for g in range(NGRP)]
                    qT = [[kv_pool.tile([128, NQ], BF16, tag=f"qT{g}_{p4}",
                                        name=f"qT{g}_{p4}")
                           for p4 in range(GP)] for g in range(NGRP)]
                    vm = [kv_pool.tile([128, 16, 2 * GP, 65], BF16,
                                       tag=f"vm{g}", name=f"vm{g}")
                          for g in range(NGRP)]
                    for g in range(NGRP):
                        nc.gpsimd.memset(vm[g][:, :, :, 64:65], 1.0)

                    # ===== stage A: LN1 (own 512 tokens) =====
                    with (
                        tc.tile_pool(name="h1p", bufs=1) as h1_pool,
                        tc.tile_pool(name="qkvw", bufs=1) as qkvw_pool,
                        tc.tile_pool(name="lnw", bufs=2) as ln_work,
                        tc.tile_pool(name="acc", bufs=1) as acc_pool,
                        tc.tile_pool(name="rep", bufs=1) as rep_pool,
                        tc.tile_pool(name="sqp", bufs=2) as sq_pool,
                        tc.tile_pool(name="psB", bufs=2, space="PSUM") as psB,
                    ):
                        h1 = [h1_pool.tile([128, NQ], BF16, tag=f"h1_{i}",
                                           name=f"h1_{i}") for i in range(CT)]
                        for ct in range(CT):
                            _dma(nc, xres[ct][:],
                                 xT[ct * 128 : (ct + 1) * 128, :])
                        # prefetch QKV weights
                        wq_t = [[None] * CT for _ in range(NGRP)]
                        wk_t = [[None] * CT for _ in range(NGRP)]
                        wv_t = [[None] * CT for _ in range(NGRP)]
                        for kind, arr, coff in (("k", wk_t, DIM),
                                                ("v", wv_t, 2 * DIM),
                                                ("q", wq_t, 0)):
                            for g in range(NGRP):
                                for ct in range(CT):
                                    rsl = slice(ct * 128, (ct + 1) * 128)
                                    arr[g][ct] = qkvw_pool.tile(
                                        [128, GW], BF16, tag=f"w{kind}{g}_{ct}",
                                        name=f"w{kind}{g}_{ct}")
                                    _dma(nc, arr[g][ct][:],
                                         qkv_wb[rsl, coff + g * GW : coff + (g + 1) * GW])

                        s_sum = acc_pool.tile([128, NQ], F32, tag="ssum",
                                              name="ssum")
                        s_sq = acc_pool.tile([128, NQ], F32, tag="ssq",
                                             name="ssq")
                        sq0 = sq_pool.tile([128, NQ], F32, tag="sq", name="sq")
                        sq1 = sq_pool.tile([128, NQ], F32, tag="sq", name="sq")
                        nc.vector.tensor_add(s_sum[:], xres[0][:], xres[1][:])
                        nc.vector.tensor_mul(sq0[:], xres[0][:], xres[0][:])
                        nc.vector.tensor_mul(sq1[:], xres[1][:], xres[1][:])
                        nc.gpsimd.tensor_add(s_sq[:], sq0[:], sq1[:])
                        for ct in range(2, CT):
                            sq = sq_pool.tile([128, NQ], F32, tag="sq", name="sq")
                            nc.vector.tensor_add(s_sum[:], s_sum[:], xres[ct][:])
                            nc.vector.tensor_mul(sq[:], xres[ct][:], xres[ct][:])
                            nc.gpsimd.tensor_add(s_sq[:], s_sq[:], sq[:])
                        nc.gpsimd.partition_all_reduce(
                            s_sq[:], s_sq[:], channels=128,
                            reduce_op=bass_isa.ReduceOp.add)
                        nc.gpsimd.partition_all_reduce(
                            s_sum[:], s_sum[:], channels=128,
                            reduce_op=bass_isa.ReduceOp.add)
                        rstd, mrs = _ln_stats_rep(
                            nc, rep_pool, s_sum, s_sq, DIM, tag="1")
                        for ct in range(CT):
                            t = ln_work.tile([128, NQ], F32, tag="lnt",
                                             name="lnt")
                            nc.vector.tensor_mul(t[:], xres[ct][:], rstd[:])
                            nc.vector.tensor_sub(t[:], t[:], mrs[:])
                            nc.scalar.activation(
                                h1[ct][:], t[:], AF.Identity,
                                scale=ln1g_t[:, ct : ct + 1],
                                bias=ln1b_t[:, ct : ct + 1])

                        # ===== stage B: QKV GEMMs (own tokens) + stage to
                        # HBM, one group-half at a time; each half's K+V is
                        # all-gathered as soon as it is staged =====
                        def emit_k(g, half):
                            gl = g % 2
                            ps = psB.tile([128, GP, NQ], F32, tag="bps",
                                          name="kps")
                            for p4 in range(GP):
                                psl = slice(p4 * 128, (p4 + 1) * 128)
                                for ct in range(CT):
                                    nc.tensor.matmul(
                                        ps[:, p4, :], wk_t[g][ct][:, psl],
                                        h1[ct][:],
                                        start=(ct == 0), stop=(ct == CT - 1))
                            ko = ln_work.tile([128, GP, NQ], BF16, tag=f"ko{g}",
                                              bufs=1, name=f"ko{g}")
                            nc.scalar.activation(ko[:], ps[:], AF.Identity)
                            _dma(nc, kv_in[half][:, gl * 1024 : (gl + 1) * 1024],
                                 ko[:].rearrange("p a b -> p (a b)"))

                        def emit_v(g, half):
                            gl = g % 2
                            ps = psB.tile([128, 4, GW], F32, tag="bps",
                                          name="vps")
                            for t4 in range(4):
                                tsl = slice(t4 * 128, (t4 + 1) * 128)
                                for ct in range(CT):
                                    nc.tensor.matmul(
                                        ps[:, t4, :], h1[ct][:, tsl],
                                        wv_t[g][ct][:],
                                        start=(ct == 0), stop=(ct == CT - 1))
                            vo = ln_work.tile([128, 4, GW], BF16, tag=f"vo{g}",
                                              bufs=1, name=f"vo{g}")
                            nc.scalar.activation(vo[:], ps[:], AF.Identity)
                            _dma(nc, kv_in[half][:, 2048 + gl * 1024 : 2048 + (gl + 1) * 1024],
                                 vo[:].rearrange("p a b -> p (a b)"))

                        for half in range(2):
                            for g in (2 * half, 2 * half + 1):
                                emit_k(g, half)
                            for g in (2 * half, 2 * half + 1):
                                emit_v(g, half)
                            nc.gpsimd.collective_compute(
                                "AllGather", ALU.bypass,
                                replica_groups=[[0, 1, 2, 3], [4, 5, 6, 7]],
                                ins=[kv_in[half].opt()],
                                outs=[kv_out[half].opt()],
                            )

                        # Q GEMMs (overlap the gathers)
                        for g in range(NGRP):
                            for p4 in range(GP):
                                p = g * GP + p4
                                psl = slice(p4 * 128, (p4 + 1) * 128)
                                ps = psB.tile([128, NQ], F32, tag="bps",
                                              name="qps")
                                for ct in range(CT):
                                    nc.tensor.matmul(
                                        ps[:], wq_t[g][ct][:, psl],
                                        h1[ct][:],
                                        start=(ct == 0), stop=(ct == CT - 1))
                                nc.scalar.activation(
                                    qT[g][p4][:], ps[:], AF.Identity,
                                    bias=qb_q[:, p : p + 1])

                        # read gathered K/V back, group-major so attention on
                        # group 0 unblocks after its own 8 DMAs
                        for g in range(NGRP):
                            half, gl = divmod(g, 2)
                            for j in range(4):
                                _dma(nc, kTm[g][:, :, j * 512 : (j + 1) * 512],
                                     kv_out[half][j, :, gl * 1024 : (gl + 1) * 1024]
                                     .rearrange("p (a b) -> p a b", a=GP))
                            for j in range(4):
                                _dma(nc,
                                     vm[g][:, j * 4 : (j + 1) * 4, :, 0:64],
                                     kv_out[half][j, :, 2048 + gl * 1024 : 2048 + (gl + 1) * 1024]
                                     .rearrange("p (a h v) -> p a h v", h=2 * GP, v=64))

                    # ---- stage C: attention, per group ----
                    with (
                        tc.tile_pool(name="psS", bufs=3, space="PSUM") as psS,
                        tc.tile_pool(name="psAV", bufs=2, space="PSUM") as psAV,
                    ):
                        for grp in range(NGRP):
                            with (
                                tc.tile_pool(name="ep", bufs=4) as e_pool,
                                tc.tile_pool(name="aup", bufs=4) as au_pool,
                            ):
                                avun = []
                                rcps = []
                                for p4 in range(GP):
                                    av2 = [psAV.tile([65, NQ], F32, tag="av",
                                                     name=f"av{h}")
                                           for h in range(2)]
                                    prev_e = None
                                    for kt in range(16):
                                        s2 = psS.tile([128, 2, NQ], F32,
                                                      tag="s", name="s")
                                        for hh in range(2):
                                            hsl = slice(hh * 64, (hh + 1) * 64)
                                            nc.tensor.matmul(
                                                s2[:, hh, :],
                                                kTm[grp][hsl, p4, kt * 128 : (kt + 1) * 128],
                                                qT[grp][p4][hsl, :],
                                                start=True, stop=True)
                                        if prev_e is not None:
                                            for hh in range(2):
                                                nc.tensor.matmul(
                                                    av2[hh][:],
                                                    vm[grp][:, kt - 1, p4 * 2 + hh, :],
                                                    prev_e[:, hh, :],
                                                    start=(kt == 1), stop=False)
                                        e2 = e_pool.tile([128, 2, NQ], BF16,
                                                         tag="e", name="e")
                                        nc.scalar.activation(e2[:], s2[:], AF.Exp)
                                        prev_e = e2
                                    for hh in range(2):
                                        nc.tensor.matmul(
                                            av2[hh][:],
                                            vm[grp][:, 15, p4 * 2 + hh, :],
                                            prev_e[:, hh, :],
                                            start=False, stop=True)
                                        au = au_pool.tile([65, NQ], F32, tag="au",
                                                          name="au")
                                        nc.vector.tensor_copy(au[:], av2[hh][:])
                                        rcp = sb_stat.tile([1, NQ], F32R, tag="rcp",
                                                           bufs=2, name="rcp")
                                        with nc.allow_low_precision("softmax 1/sum"):
                                            nc.vector.reciprocal(rcp[:], au[64:65, :])
                                        avun.append(au)
                                        rcps.append(rcp)
                                for i in range(2 * GP):
                                    p4, hh = divmod(i, 2)
                                    p = grp * GP + p4
                                    hsl = slice(hh * 64, (hh + 1) * 64)
                                    rb = sb_stat.tile([64, NQ], F32R, tag="rb",
                                                      bufs=3, name="rb")
                                    nc.gpsimd.partition_broadcast(rb[:], rcps[i][:])
                                    nc.vector.tensor_mul(yT[p][hsl, :],
                                                         avun[i][0:64, :], rb[:])

                # ========== stage D+E: proj + residual + LN2 ================
                with tc.tile_pool(name="x2p", bufs=CT) as x2_pool:
                    x2 = [x2_pool.tile([128, NQ], F32R, tag="x2", name=f"x2_{i}")
                          for i in range(CT)]
                    with tc.tile_pool(name="h2p", bufs=CT) as h2_pool:
                        h2 = [h2_pool.tile([128, NQ], BF16, tag="h2",
                                           name=f"h2_{i}") for i in range(CT)]
                        with (
                            tc.tile_pool(name="pwp", bufs=CT) as pw_pool,
                            tc.tile_pool(name="psD", bufs=2, space="PSUM") as psD,
                            tc.tile_pool(name="acc2", bufs=1) as acc2_pool,
                            tc.tile_pool(name="rep2", bufs=1) as rep2_pool,
                            tc.tile_pool(name="sq2p", bufs=2) as sq2_pool,
                            tc.tile_pool(name="lnw2", bufs=2) as ln_work2,
                        ):
                            pw_t = [pw_pool.tile([128, DIM], BF16, tag="pw",
                                                 name=f"pw{i}") for i in range(CT)]
                            for ct in range(CT):
                                _dma(nc, pw_t[ct][:],
                                     proj_wb[ct * 128 : (ct + 1) * 128, :])
                            s2_sum = acc2_pool.tile([128, NQ], F32, tag="s2sum",
                                                    name="s2sum")
                            s2_sq = acc2_pool.tile([128, NQ], F32, tag="s2sq",
                                                   name="s2sq")
                            for co in range(CT):
                                ps = psD.tile([128, NQ], F32, tag="dps", name="dps")
                                for ct in range(CT):
                                    nc.tensor.matmul(
                                        ps[:], pw_t[ct][:, co * 128 : (co + 1) * 128],
                                        yT[ct][:],
                                        start=(ct == 0), stop=(ct == CT - 1))
                                nc.vector.scalar_tensor_tensor(
                                    x2[co][:], ps[:], projb_t[:, co : co + 1],
                                    xres[co][:], op0=ALU.add, op1=ALU.add)
                                if co == 0:
                                    nc.vector.tensor_copy(s2_sum[:], x2[0][:])
                                    nc.gpsimd.tensor_mul(s2_sq[:], x2[0][:], x2[0][:])
                                else:
                                    sq = sq2_pool.tile([128, NQ], F32R, tag="sq2",
                                                       name="sq2")
                                    nc.vector.tensor_add(s2_sum[:], s2_sum[:],
                                                         x2[co][:])
                                    nc.gpsimd.tensor_mul(sq[:], x2[co][:], x2[co][:])
                                    nc.gpsimd.tensor_add(s2_sq[:], s2_sq[:], sq[:])
                            nc.gpsimd.partition_all_reduce(
                                s2_sq[:], s2_sq[:], channels=128,
                                reduce_op=bass_isa.ReduceOp.add)
                            nc.gpsimd.partition_all_reduce(
                                s2_sum[:], s2_sum[:], channels=128,
                                reduce_op=bass_isa.ReduceOp.add)
                            rstd2, mrs2 = _ln_stats_rep(
                                nc, rep2_pool, s2_sum, s2_sq, DIM, tag="2")
                            for ct in range(CT):
                                t = ln_work2.tile([128, NQ], F32, tag="lnt2",
                                                  name="lnt2")
                                nc.vector.tensor_mul(t[:], x2[ct][:], rstd2[:])
                                nc.vector.tensor_sub(t[:], t[:], mrs2[:])
                                nc.vector.tensor_scalar(
                                    h2[ct][:], t[:],
                                    ln2g_t[:, ct : ct + 1], ln2b_t[:, ct : ct + 1],
                                    op0=ALU.mult, op1=ALU.add)

                        # ============ stage F: MLP ==============================
                        with tc.tile_pool(name="gp", bufs=FT) as g_pool:
                            g_t = [g_pool.tile([128, NQ], BF16, tag="g",
                                               name=f"g{i}") for i in range(FT)]
                            with (
                                tc.tile_pool(name="w1p", bufs=8) as w1_pool,
                                tc.tile_pool(name="psF1", bufs=8, space="PSUM") as psF1,
                            ):
                                for fog in range(8):
                                    w1_t = [w1_pool.tile([128, 512], BF16, tag="w1",
                                                         name=f"w1_{i}")
                                            for i in range(CT)]
                                    for ct in range(CT):
                                        _dma(nc, w1_t[ct][:],
                                             fc1_wb[ct * 128 : (ct + 1) * 128,
                                                    fog * 512 : (fog + 1) * 512])
                                    pss = [psF1.tile([128, NQ], F32, tag="f1ps",
                                                     name=f"f1ps{i}")
                                           for i in range(4)]
                                    for ct in range(CT):
                                        for fo4 in range(4):
                                            nc.tensor.matmul(
                                                pss[fo4][:],
                                                w1_t[ct][:, fo4 * 128 : (fo4 + 1) * 128],
                                                h2[ct][:],
                                                start=(ct == 0), stop=(ct == CT - 1))
                                    for fo4 in range(4):
                                        fo = fog * 4 + fo4
                                        nc.scalar.activation(
                                            g_t[fo][:], pss[fo4][:],
                                            GELU_AF or AF.Gelu,
                                            bias=fc1b_t[:, fo : fo + 1])
                            with (
                                tc.tile_pool(name="w2p", bufs=8) as w2_pool,
                                tc.tile_pool(name="psF2", bufs=8, space="PSUM") as psF2,
                                tc.tile_pool(name="op", bufs=4) as out_pool,
                            ):
                                for cog in range(2):
                                    pss = [psF2.tile([128, NQ], F32, tag="f2ps",
                                                     name=f"f2ps{i}")
                                           for i in range(4)]
                                    for ko in range(FT):
                                        w2_t = w2_pool.tile([128, 512], BF16, tag="w2")
                                        _dma(nc, w2_t[:],
                                             fc2_wb[ko * 128 : (ko + 1) * 128,
                                                    cog * 512 : (cog + 1) * 512])
                                        for co4 in range(4):
                                            nc.tensor.matmul(
                                                pss[co4][:],
                                                w2_t[:, co4 * 128 : (co4 + 1) * 128],
                                                g_t[ko][:],
                                                start=(ko == 0), stop=(ko == FT - 1))
                                    for co4 in range(4):
                                        co = cog * 4 + co4
                                        o_t = out_pool.tile([128, NQ], F32, tag="o")
                                        nc.vector.scalar_tensor_tensor(
                                            o_t[:], pss[co4][:], fc2b_t[:, co : co + 1],
                                            x2[co][:], op0=ALU.add, op1=ALU.add)
                                        _dma(nc, outT[co * 128 : (co + 1) * 128, :],
                                             o_t[:])

    nc.compile()
    return nc


_CACHED_NC = None


def _get_nc():
    global _CACHED_NC
    if _CACHED_NC is None:
        _CACHED_NC = build_program()
    return _CACHED_NC


def make_in_maps(inputs):
    import ml_dtypes

    bf16 = ml_dtypes.bfloat16
    ins = {k: np.ascontiguousarray(np.asarray(v), dtype=np.float32)
           for k, v in inputs.items()}
    qkv_wb = np.ascontiguousarray(ins["qkv_w"].astype(bf16))
    proj_wb = np.ascontiguousarray(ins["proj_w"].astype(bf16))
    fc1_wb = np.ascontiguousarray(ins["fc1_w"].astype(bf16))
    fc2_wb = np.ascontiguousarray(ins["fc2_w"].astype(bf16))
    proj_b_eff = (ins["proj_b"]
                  + ins["qkv_b"][2048:].astype(np.float64)
                  @ ins["proj_w"].astype(np.float64)).astype(np.float32)
    cols = [ins["ln1_g"], ins["ln1_b"], ins["ln2_g"], ins["ln2_b"],
            proj_b_eff, ins["fc2_b"], ins["fc1_b"],
            ins["qkv_b"][:1024], ins["qkv_b"][1024:2048]]
    packed = np.concatenate(
        [c.reshape(-1, 128).T for c in cols] + [np.ones((128, 8), np.float32)],
        axis=1)
    packed = np.ascontiguousarray(packed)
    in_maps = []
    for core in range(N_CORES):
        b = core // 4
        qs = (core % 4) * NQ
        xt = np.ascontiguousarray(ins["x"][b][qs : qs + NQ].T)
        in_maps.append({
            "xT": xt,
            "bias_pack": packed,
            "qkv_wb": qkv_wb, "proj_wb": proj_wb,
            "fc1_wb": fc1_wb, "fc2_wb": fc2_wb,
        })
    return in_maps


def gather_output(results):
    out = np.empty((2, NTOK, DIM), dtype=np.float32)
    for core in range(N_CORES):
        b = core // 4
        qs = (core % 4) * NQ
        out[b, qs : qs + NQ, :] = results[core]["outT"].T
    return out


def kernel(**inputs) -> np.ndarray:
    nc = _get_nc()
    in_maps = make_in_maps(inputs)
    res = run_bass_kernel_spmd(nc, in_maps, list(range(N_CORES)))
    return gather_output(res.results)


# revision 14
# speedup vs baseline: 184.4956x; 184.4956x over previous
"""Trainium2 Bass kernel for a dense pre-norm transformer block (v2: K/V all-gather).

Reference computation (fp32):
    h = LN1(x); qkv = h @ qkv_w + qkv_b; attention (16 heads, no 1/sqrt(d));
    x = x + attn_out @ proj_w + proj_b;
    h2 = LN2(x); x = x + gelu_exact(h2 @ fc1_w + fc1_b) @ fc2_w + fc2_b

Shapes: x [2, 2048, 1024], heads 16 x 64, MLP 4096.

Sharding (8 NeuronCores):
    cores 0-3 -> batch 0, cores 4-7 -> batch 1. Each core owns 512 query
    tokens (x input = just those). LN1 + Q/K/V are computed for the OWN 512
    tokens only; K and V (bf16, 2MB) are then AllGather'd across the 4 cores
    of the batch (staged through HBM). Keys land in global token order --
    attention is permutation-invariant over keys so no rotation is needed;
    the program is SPMD-uniform because even the own K/V block is read back
    from the gather output. Attention/proj/MLP run on the own 512 queries.

Design notes (carried from v1):
    - activations feature-major [C, tokens] on-chip; GEMM operands bf16.
    - LN stats via DVE sums + GpSimd partition_all_reduce; softmax 1/sum via
      GpSimd partition_broadcast.
    - softmax skips max-subtraction (scores ~N(0,3.3^2), fp32 exp safe).
    - K bias dropped (softmax-invariant); V bias folded into proj bias.
"""

import sys

if "/opt/trn_rl_repo" not in sys.path:
    sys.path.insert(0, "/opt/trn_rl_repo")

import numpy as np

import concourse.bass as bass
import concourse.bass_isa as bass_isa
import concourse.mybir as mybir
import concourse.tile as tile
from concourse import bacc
from concourse.bass_utils import run_bass_kernel_spmd

F32 = mybir.dt.float32
F32R = mybir.dt.float32r
BF16 = mybir.dt.bfloat16
AF = mybir.ActivationFunctionType
ALU = mybir.AluOpType

DIM = 1024
CT = DIM // 128          # 8 feature tiles
NTOK = 2048              # tokens per batch
NQ = 512                 # query tokens per core
H = 16
D = 64
MLP = 4096
FT = MLP // 128          # 32 mlp feature tiles
EPS = 1e-5
N_CORES = 8
GELU_AF = None
NGRP = 4                 # attention processed in 4 groups of 2 head-pairs
GP = 2                   # head pairs per group (256 qkv columns)
GW = GP * 128            # qkv columns per group


def _dma(nc, out, in_):
    nc.sync.dma_start(out=out, in_=in_)


def _ln_stats_rep(nc, pool, ar_sum, ar_sq, ntok_norm, tag):
    """From all-reduced (partition-replicated) sum / sum-of-squares tiles
    [128, NQ] produce replicated rstd and mean*rstd tiles [128, NQ]."""
    mean = pool.tile([128, NQ], F32, tag=f"mean{tag}", bufs=1, name="mean")
    m2 = pool.tile([128, NQ], F32, tag=f"m2{tag}", bufs=1, name="m2")
    nc.scalar.activation(mean[:], ar_sum[:], AF.Identity, scale=1.0 / ntok_norm)
    nc.scalar.activation(m2[:], mean[:], AF.Square)
    nc.vector.scalar_tensor_tensor(
        m2[:], ar_sq[:], 1.0 / ntok_norm, m2[:],
        op0=ALU.mult, op1=ALU.subtract)
    nc.vector.tensor_scalar_add(m2[:], m2[:], EPS)
    nc.vector.reciprocal(m2[:], m2[:])
    rstd = pool.tile([128, NQ], BF16, tag=f"rstd{tag}", bufs=2, name="rstd")
    mrs = pool.tile([128, NQ], BF16, tag=f"mrs{tag}", bufs=2, name="mrs")
    nc.scalar.activation(rstd[:], m2[:], AF.Sqrt)
    nc.vector.tensor_mul(mrs[:], mean[:], rstd[:])
    return rstd, mrs


def build_program(reps=1):
    nc = bacc.Bacc("TRN2", target_bir_lowering=False)

    xT = nc.declare_dram_parameter("xT", [DIM, NQ], F32R, isOutput=False)
    qkv_wb = nc.declare_dram_parameter("qkv_wb", [DIM, 3 * DIM], BF16, isOutput=False)
    proj_wb = nc.declare_dram_parameter("proj_wb", [DIM, DIM], BF16, isOutput=False)
    fc1_wb = nc.declare_dram_parameter("fc1_wb", [DIM, MLP], BF16, isOutput=False)
    fc2_wb = nc.declare_dram_parameter("fc2_wb", [MLP, DIM], BF16, isOutput=False)
    bias_pack = nc.declare_dram_parameter("bias_pack", [128, 104], F32, isOutput=False)
    outT = nc.declare_dram_parameter("outT", [DIM, NQ], F32, isOutput=True)

    with tile.TileContext(nc, pool_alloc_mode="queue") as tc:
      for _rep in range(reps):
            with (
                tc.tile_pool(name="const", bufs=1) as const,
                tc.tile_pool(name="xres", bufs=CT) as xres_pool,
                tc.tile_pool(name="yT", bufs=CT) as yT_pool,
                tc.tile_pool(name="pwp", bufs=CT) as pw_pool,
                tc.tile_pool(name="stat", bufs=1) as sb_stat,
                tc.tile_pool(name="dram", bufs=1, space="DRAM") as dram,
            ):
                bp = const.tile([128, 104], F32, tag="bp")
                _dma(nc, bp[:], bias_pack[:, :])
                ln1g_t = bp[:, 0:8]
                ln1b_t = bp[:, 8:16]
                ln2g_t = bp[:, 16:24]
                ln2b_t = bp[:, 24:32]
                projb_t = bp[:, 32:40]
                fc2b_t = bp[:, 40:48]
                fc1b_t = bp[:, 48:80]
                qb_q = bp[:, 80:88]

                xres = [xres_pool.tile([128, NQ], F32R, tag="xres",
                                       name=f"xres{i}") for i in range(CT)]
                yT = [yT_pool.tile([128, NQ], BF16, tag="yT", name=f"yT{i}")
                      for i in range(CT)]

                # staging DRAM for the K/V all-gather, split in two halves
                # (groups 0-1 / groups 2-3) so attention on the first half
                # overlaps the second collective. Per half: cols [0:2048) K
                # ([128, GP, 512] per group), [2048:4096) V ([128,4,4,64]).
                kv_in = [dram.tile([128, 4096], BF16, name=f"kv_in{h}")
                         for h in range(2)]
                kv_out = [dram.tile([4, 128, 4096], BF16, name=f"kv_out{h}")
                          for h in range(2)]

                with (
                    tc.tile_pool(name="kvp", bufs=1) as kv_pool,
                ):
                    kTm = [kv_pool.tile([128, GP, NTOK], BF16, tag=f"kTm{g}",
                                        name=f"kTm{g}") for g in range(NGRP)]
                    qT = [[kv_pool.tile([128, NQ], BF16, tag=f"qT{g}_{p4}",
                                        name=f"qT{g}_{p4}")
                           for p4 in range(GP)] for g in range(NGRP)]
                    vm = [kv_pool.tile([128, 16, 2 * GP, 65], BF16,
                                       tag=f"vm{g}", name=f"vm{g}")
                          for g in range(NGRP)]
                    for g in range(NGRP):
                        nc.gpsimd.memset(vm[g][:, :, :, 64:65], 1.0)

                    # ===== stage A: LN1 (own 512 tokens) =====
                    with (
                        tc.tile_pool(name="h1p", bufs=1) as h1_pool,
                        tc.tile_pool(name="qkvw", bufs=1) as qkvw_pool,
                        tc.tile_pool(name="lnw", bufs=2) as ln_work,
                        tc.tile_pool(name="acc", bufs=1) as acc_pool,
                        tc.tile_pool(name="rep", bufs=1) as rep_pool,
                        tc.tile_pool(name="sqp", bufs=2) as sq_pool,
                        tc.tile_pool(name="psB", bufs=2, space="PSUM") as psB,
                    ):
                        h1 = [h1_pool.tile([128, NQ], BF16, tag=f"h1_{i}",
                                           name=f"h1_{i}") for i in range(CT)]
                        for ct in range(CT):
                            _dma(nc, xres[ct][:],
                                 xT[ct * 128 : (ct + 1) * 128, :])
                        # prefetch QKV weights
                        wq_t = [[None] * CT for _ in range(NGRP)]
                        wk_t = [[None] * CT for _ in range(NGRP)]
                        wv_t = [[None] * CT for _ in range(NGRP)]
                        ring = {"k": 16, "v": 12, "q": 8}
                        for kind, arr, coff in (("k", wk_t, DIM),
                                                ("v", wv_t, 2 * DIM),
                                                ("q", wq_t, 0)):
                            for g in range(NGRP):
                                for ct in range(CT):
                                    rsl = slice(ct * 128, (ct + 1) * 128)
                                    arr[g][ct] = qkvw_pool.tile(
                                        [128, GW], BF16, tag=f"w{kind}",
                                        bufs=ring[kind], name=f"w{kind}{g}_{ct}")
                                    _dma(nc, arr[g][ct][:],
                                         qkv_wb[rsl, coff + g * GW : coff + (g + 1) * GW])

                        s_sum = acc_pool.tile([128, NQ], F32, tag="ssum",
                                              name="ssum")
                        s_sq = acc_pool.tile([128, NQ], F32, tag="ssq",
                                             name="ssq")
                        sq0 = sq_pool.tile([128, NQ], F32, tag="sq", name="sq")
                        sq1 = sq_pool.tile([128, NQ], F32, tag="sq", name="sq")
                        nc.vector.tensor_add(s_sum[:], xres[0][:], xres[1][:])
                        nc.vector.tensor_mul(sq0[:], xres[0][:], xres[0][:])
                        nc.vector.tensor_mul(sq1[:], xres[1][:], xres[1][:])
                        nc.gpsimd.tensor_add(s_sq[:], sq0[:], sq1[:])
                        for ct in range(2, CT):
                            sq = sq_pool.tile([128, NQ], F32, tag="sq", name="sq")
                            nc.vector.tensor_add(s_sum[:], s_sum[:], xres[ct][:])
                            nc.vector.tensor_mul(sq[:], xres[ct][:], xres[ct][:])
                            nc.gpsimd.tensor_add(s_sq[:], s_sq[:], sq[:])
                        nc.gpsimd.partition_all_reduce(
                            s_sq[:], s_sq[:], channels=128,
                            reduce_op=bass_isa.ReduceOp.add)
                        nc.gpsimd.partition_all_reduce(
                            s_sum[:], s_sum[:], channels=128,
                            reduce_op=bass_isa.ReduceOp.add)
                        rstd, mrs = _ln_stats_rep(
                            nc, rep_pool, s_sum, s_sq, DIM, tag="1")
                        for ct in range(CT):
                            t = ln_work.tile([128, NQ], F32, tag="lnt",
                                             name="lnt")
                            nc.vector.tensor_mul(t[:], xres[ct][:], rstd[:])
                            nc.vector.tensor_sub(t[:], t[:], mrs[:])
                            nc.scalar.activation(
                                h1[ct][:], t[:], AF.Identity,
                                scale=ln1g_t[:, ct : ct + 1],
                                bias=ln1b_t[:, ct : ct + 1])

                        # ===== stage B: QKV GEMMs (own tokens) + stage to
                        # HBM, one group-half at a time; each half's K+V is
                        # all-gathered as soon as it is staged =====
                        def emit_k(g, half):
                            gl = g % 2
                            ps = psB.tile([128, GP, NQ], F32, tag="bps",
                                          name="kps")
                            for p4 in range(GP):
                                psl = slice(p4 * 128, (p4 + 1) * 128)
                                for ct in range(CT):
                                    nc.tensor.matmul(
                                        ps[:, p4, :], wk_t[g][ct][:, psl],
                                        h1[ct][:],
                                        start=(ct == 0), stop=(ct == CT - 1))
                            ko = ln_work.tile([128, GP, NQ], BF16, tag="ko",
                                              bufs=2, name=f"ko{g}")
                            nc.scalar.activation(ko[:], ps[:], AF.Identity)
                            _dma(nc, kv_in[half][:, gl * 1024 : (gl + 1) * 1024],
                                 ko[:].rearrange("p a b -> p (a b)"))

                        def emit_v(g, half):
                            gl = g % 2
                            ps = psB.tile([128, 4, GW], F32, tag="bps",
                                          name="vps")
                            for t4 in range(4):
                                tsl = slice(t4 * 128, (t4 + 1) * 128)
                                for ct in range(CT):
                                    nc.tensor.matmul(
                                        ps[:, t4, :], h1[ct][:, tsl],
                                        wv_t[g][ct][:],
                                        start=(ct == 0), stop=(ct == CT - 1))
                            vo = ln_work.tile([128, 4, GW], BF16, tag="vo",
                                              bufs=2, name=f"vo{g}")
                            nc.scalar.activation(vo[:], ps[:], AF.Identity)
                            _dma(nc, kv_in[half][:, 2048 + gl * 1024 : 2048 + (gl + 1) * 1024],
                                 vo[:].rearrange("p a b -> p (a b)"))

                        for half in range(2):
                            for g in (2 * half, 2 * half + 1):
                                emit_k(g, half)
                            for g in (2 * half, 2 * half + 1):
                                emit_v(g, half)
                            nc.gpsimd.collective_compute(
                                "AllGather", ALU.bypass,
                                replica_groups=[[0, 1, 2, 3], [4, 5, 6, 7]],
                                ins=[kv_in[half].opt()],
                                outs=[kv_out[half].opt()],
                            )

                        # Q GEMMs (overlap the gathers)
                        for g in range(NGRP):
                            for p4 in range(GP):
                                p = g * GP + p4
                                psl = slice(p4 * 128, (p4 + 1) * 128)
                                ps = psB.tile([128, NQ], F32, tag="bps",
                                              name="qps")
                                for ct in range(CT):
                                    nc.tensor.matmul(
                                        ps[:], wq_t[g][ct][:, psl],
                                        h1[ct][:],
                                        start=(ct == 0), stop=(ct == CT - 1))
                                nc.scalar.activation(
                                    qT[g][p4][:], ps[:], AF.Identity,
                                    bias=qb_q[:, p : p + 1])

                        # prefetch proj weights under the gather (SP idle)
                        pw_t = [pw_pool.tile([128, DIM], BF16, tag="pw",
                                             name=f"pw{i}") for i in range(CT)]
                        for ct in range(CT):
                            _dma(nc, pw_t[ct][:],
                                 proj_wb[ct * 128 : (ct + 1) * 128, :])

                        # read gathered K/V back, group-major so attention on
                        # group 0 unblocks after its own 8 DMAs
                        for g in range(NGRP):
                            half, gl = divmod(g, 2)
                            for j in range(4):
                                _dma(nc, kTm[g][:, :, j * 512 : (j + 1) * 512],
                                     kv_out[half][j, :, gl * 1024 : (gl + 1) * 1024]
                                     .rearrange("p (a b) -> p a b", a=GP))
                            for j in range(4):
                                _dma(nc,
                                     vm[g][:, j * 4 : (j + 1) * 4, :, 0:64],
                                     kv_out[half][j, :, 2048 + gl * 1024 : 2048 + (gl + 1) * 1024]
                                     .rearrange("p (a h v) -> p a h v", h=2 * GP, v=64))

                    # ---- stage C: attention, per group ----
                    with (
                        tc.tile_pool(name="psS", bufs=3, space="PSUM") as psS,
                        tc.tile_pool(name="psAV", bufs=2, space="PSUM") as psAV,
                    ):
                        for grp in range(NGRP):
                            with (
                                tc.tile_pool(name="ep", bufs=4) as e_pool,
                                tc.tile_pool(name="aup", bufs=4) as au_pool,
                            ):
                                avun = []
                                rcps = []
                                for p4 in range(GP):
                                    av2 = [psAV.tile([65, NQ], F32, tag="av",
                                                     name=f"av{h}")
                                           for h in range(2)]
                                    prev_e = None
                                    for kt in range(16):
                                        s2 = psS.tile([128, 2, NQ], F32,
                                                      tag="s", name="s")
                                        for hh in range(2):
                                            hsl = slice(hh * 64, (hh + 1) * 64)
                                            nc.tensor.matmul(
                                                s2[:, hh, :],
                                                kTm[grp][hsl, p4, kt * 128 : (kt + 1) * 128],
                                                qT[grp][p4][hsl, :],
                                                start=True, stop=True)
                                        if prev_e is not None:
                                            for hh in range(2):
                                                nc.tensor.matmul(
                                                    av2[hh][:],
                                                    vm[grp][:, kt - 1, p4 * 2 + hh, :],
                                                    prev_e[:, hh, :],
                                                    start=(kt == 1), stop=False)
                                        e2 = e_pool.tile([128, 2, NQ], BF16,
                                                         tag="e", name="e")
                                        nc.scalar.activation(e2[:], s2[:], AF.Exp)
                                        prev_e = e2
                                    for hh in range(2):
                                        nc.tensor.matmul(
                                            av2[hh][:],
                                            vm[grp][:, 15, p4 * 2 + hh, :],
                                            prev_e[:, hh, :],
                                            start=False, stop=True)
                                        au = au_pool.tile([65, NQ], F32, tag="au",
                                                          name="au")
                                        nc.vector.tensor_copy(au[:], av2[hh][:])
                                        rcp = sb_stat.tile([1, NQ], F32R, tag="rcp",
                                                           bufs=2, name="rcp")
                                        with nc.allow_low_precision("softmax 1/sum"):
                                            nc.vector.reciprocal(rcp[:], au[64:65, :])
                                        avun.append(au)
                                        rcps.append(rcp)
                                for i in range(2 * GP):
                                    p4, hh = divmod(i, 2)
                                    p = grp * GP + p4
                                    hsl = slice(hh * 64, (hh + 1) * 64)
                                    rb = sb_stat.tile([64, NQ], F32R, tag="rb",
                                                      bufs=3, name="rb")
                                    nc.gpsimd.partition_broadcast(rb[:], rcps[i][:])
                                    nc.vector.tensor_mul(yT[p][hsl, :],
                                                         avun[i][0:64, :], rb[:])

                # ========== stage D+E: proj + residual + LN2 ================
                with tc.tile_pool(name="x2p", bufs=CT) as x2_pool:
                    x2 = [x2_pool.tile([128, NQ], F32R, tag="x2", name=f"x2_{i}")
                          for i in range(CT)]
                    with tc.tile_pool(name="h2p", bufs=CT) as h2_pool:
                        h2 = [h2_pool.tile([128, NQ], BF16, tag="h2",
                                           name=f"h2_{i}") for i in range(CT)]
                        with (
                            tc.tile_pool(name="psD", bufs=2, space="PSUM") as psD,
                            tc.tile_pool(name="acc2", bufs=1) as acc2_pool,
                            tc.tile_pool(name="rep2", bufs=1) as rep2_pool,
                            tc.tile_pool(name="sq2p", bufs=2) as sq2_pool,
                            tc.tile_pool(name="lnw2", bufs=2) as ln_work2,
                        ):
                            s2_sum = acc2_pool.tile([128, NQ], F32, tag="s2sum",
                                                    name="s2sum")
                            s2_sq = acc2_pool.tile([128, NQ], F32, tag="s2sq",
                                                   name="s2sq")
                            for co in range(CT):
                                ps = psD.tile([128, NQ], F32, tag="dps", name="dps")
                                for ct in range(CT):
                                    nc.tensor.matmul(
                                        ps[:], pw_t[ct][:, co * 128 : (co + 1) * 128],
                                        yT[ct][:],
                                        start=(ct == 0), stop=(ct == CT - 1))
                                nc.vector.scalar_tensor_tensor(
                                    x2[co][:], ps[:], projb_t[:, co : co + 1],
                                    xres[co][:], op0=ALU.add, op1=ALU.add)
                                if co == 0:
                                    nc.vector.tensor_copy(s2_sum[:], x2[0][:])
                                    nc.gpsimd.tensor_mul(s2_sq[:], x2[0][:], x2[0][:])
                                else:
                                    sq = sq2_pool.tile([128, NQ], F32R, tag="sq2",
                                                       name="sq2")
                                    nc.vector.tensor_add(s2_sum[:], s2_sum[:],
                                                         x2[co][:])
                                    nc.gpsimd.tensor_mul(sq[:], x2[co][:], x2[co][:])
                                    nc.gpsimd.tensor_add(s2_sq[:], s2_sq[:], sq[:])
                            nc.gpsimd.partition_all_reduce(
                                s2_sq[:], s2_sq[:], channels=128,
                                reduce_op=bass_isa.ReduceOp.add)
                            nc.gpsimd.partition_all_reduce(
                                s2_sum[:], s2_sum[:], channels=128,
                                reduce_op=bass_isa.ReduceOp.add)
                            rstd2, mrs2 = _ln_stats_rep(
                                nc, rep2_pool, s2_sum, s2_sq, DIM, tag="2")
                            for ct in range(CT):
                                t = ln_work2.tile([128, NQ], F32, tag="lnt2",
                                                  name="lnt2")
                                nc.vector.tensor_mul(t[:], x2[ct][:], rstd2[:])
                                nc.vector.tensor_sub(t[:], t[:], mrs2[:])
                                nc.vector.tensor_scalar(
                                    h2[ct][:], t[:],
                                    ln2g_t[:, ct : ct + 1], ln2b_t[:, ct : ct + 1],
                                    op0=ALU.mult, op1=ALU.add)

                        # ============ stage F: MLP ==============================
                        with tc.tile_pool(name="gp", bufs=FT) as g_pool:
                            g_t = [g_pool.tile([128, NQ], BF16, tag="g",
                                               name=f"g{i}") for i in range(FT)]
                            with (
                                tc.tile_pool(name="w1p", bufs=8) as w1_pool,
                                tc.tile_pool(name="psF1", bufs=8, space="PSUM") as psF1,
                            ):
                                for fog in range(8):
                                    w1_t = [w1_pool.tile([128, 512], BF16, tag="w1",
                                                         name=f"w1_{i}")
                                            for i in range(CT)]
                                    for ct in range(CT):
                                        _dma(nc, w1_t[ct][:],
                                             fc1_wb[ct * 128 : (ct + 1) * 128,
                                                    fog * 512 : (fog + 1) * 512])
                                    pss = [psF1.tile([128, NQ], F32, tag="f1ps",
                                                     name=f"f1ps{i}")
                                           for i in range(4)]
                                    for ct in range(CT):
                                        for fo4 in range(4):
                                            nc.tensor.matmul(
                                                pss[fo4][:],
                                                w1_t[ct][:, fo4 * 128 : (fo4 + 1) * 128],
                                                h2[ct][:],
                                                start=(ct == 0), stop=(ct == CT - 1))
                                    for fo4 in range(4):
                                        fo = fog * 4 + fo4
                                        nc.scalar.activation(
                                            g_t[fo][:], pss[fo4][:],
                                            GELU_AF or AF.Gelu,
                                            bias=fc1b_t[:, fo : fo + 1])
                            with (
                                tc.tile_pool(name="w2p", bufs=8) as w2_pool,
                                tc.tile_pool(name="psF2", bufs=8, space="PSUM") as psF2,
                                tc.tile_pool(name="op", bufs=4) as out_pool,
                            ):
                                for cog in range(2):
                                    pss = [psF2.tile([128, NQ], F32, tag="f2ps",
                                                     name=f"f2ps{i}")
                                           for i in range(4)]
                                    for ko in range(FT):
                                        w2_t = w2_pool.tile([128, 512], BF16, tag="w2")
                                        _dma(nc, w2_t[:],
                                             fc2_wb[ko * 128 : (ko + 1) * 128,
                                                    cog * 512 : (cog + 1) * 512])
                                        for co4 in range(4):
                                            nc.tensor.matmul(
                                                pss[co4][:],
                                                w2_t[:, co4 * 128 : (co4 + 1) * 128],
                                                g_t[ko][:],
                                                start=(ko == 0), stop=(ko == FT - 1))
                                    for co4 in range(4):
                                        co = cog * 4 + co4
                                        o_t = out_pool.tile([128, NQ], F32, tag="o")
                                        nc.vector.scalar_tensor_tensor(
                                            o_t[:], pss[co4][:], fc2b_t[:, co : co + 1],
                                            x2[co][:], op0=ALU.add, op1=ALU.add)
                                        _dma(nc, outT[co * 128 : (co + 1) * 128, :],
                                             o_t[:])

    nc.compile()
    return nc


_CACHED_NC = None


def _get_nc():
    global _CACHED_NC
    if _CACHED_NC is None:
        _CACHED_NC = build_program()
    return _CACHED_NC


def make_in_maps(inputs):
    import ml_dtypes

    bf16 = ml_dtypes.bfloat16
    ins = {k: np.ascontiguousarray(np.asarray(v), dtype=np.float32)
           for k, v in inputs.items()}
    qkv_wb = np.ascontiguousarray(ins["qkv_w"].astype(bf16))
    proj_wb = np.ascontiguousarray(ins["proj_w"].astype(bf16))
    fc1_wb = np.ascontiguousarray(ins["fc1_w"].astype(bf16))
    fc2_wb = np.ascontiguousarray(ins["fc2_w"].astype(bf16))
    proj_b_eff = (ins["proj_b"]
                  + ins["qkv_b"][2048:].astype(np.float64)
                  @ ins["proj_w"].astype(np.float64)).astype(np.float32)
    cols = [ins["ln1_g"], ins["ln1_b"], ins["ln2_g"], ins["ln2_b"],
            proj_b_eff, ins["fc2_b"], ins["fc1_b"],
            ins["qkv_b"][:1024], ins["qkv_b"][1024:2048]]
    packed = np.concatenate(
        [c.reshape(-1, 128).T for c in cols] + [np.ones((128, 8), np.float32)],
        axis=1)
    packed = np.ascontiguousarray(packed)
    in_maps = []
    for core in range(N_CORES):
        b = core // 4
        qs = (core % 4) * NQ
        xt = np.ascontiguousarray(ins["x"][b][qs : qs + NQ].T)
        in_maps.append({
            "xT": xt,
            "bias_pack": packed,
            "qkv_wb": qkv_wb, "proj_wb": proj_wb,
            "fc1_wb": fc1_wb, "fc2_wb": fc2_wb,
        })
    return in_maps


def gather_output(results):
    out = np.empty((2, NTOK, DIM), dtype=np.float32)
    for core in range(N_CORES):
        b = core // 4
        qs = (core % 4) * NQ
        out[b, qs : qs + NQ, :] = results[core]["outT"].T
    return out


def kernel(**inputs) -> np.ndarray:
    nc = _get_nc()
    in_maps = make_in_maps(inputs)
    res = run_bass_kernel_spmd(nc, in_maps, list(range(N_CORES)))
    return gather_output(res.results)
